# revision 1
# baseline (speedup 1.0000x reference)
import sys

import numpy as np

sys.path.insert(0, "/opt/trn_rl_repo")

B, H, S, F, D = 16, 8, 512, 512, 64
TOPK = 51
LN_EPS = 1e-5
NCORES = 8
BPC = B // NCORES  # batches per core

_cache = {}
last_result = None  # test.py can inspect exec_time_ns / traces


def _build_nc():
    """SPMD program (same on all 8 cores): per (b,h) unit compute
    outT[d, s] = sum_f v[f, d] * attnT[f, s] with PSUM accumulation
    over four 128-row f chunks."""
    from contextlib import ExitStack

    import concourse.mybir as mybir
    import concourse.tile as tile
    from concourse import bacc
    from concourse.bass import ds

    nc = bacc.Bacc(
        "TRN2",
        target_bir_lowering=False,
        debug=False,
        num_devices=NCORES,
    )
    f32 = mybir.dt.float32
    at_d = nc.dram_tensor("at", [BPC * H * F, S], f32, kind="ExternalInput").ap()
    v_d = nc.dram_tensor("v", [BPC * H * F, D], f32, kind="ExternalInput").ap()
    o_d = nc.dram_tensor("o", [BPC * H * D, S], f32, kind="ExternalOutput").ap()

    with tile.TileContext(nc) as tc, ExitStack() as ctx:
        a_pool = ctx.enter_context(tc.tile_pool(name="a", bufs=8))
        v_pool = ctx.enter_context(tc.tile_pool(name="vv", bufs=8))
        p_pool = ctx.enter_context(tc.tile_pool(name="ps", bufs=6, space="PSUM"))
        o_pool = ctx.enter_context(tc.tile_pool(name="oo", bufs=4))
        KF = F // 128
        for u in range(BPC * H):
            psum = p_pool.tile([D, S], f32)
            for kf in range(KF):
                at_t = a_pool.tile([128, S], f32)
                nc.gpsimd.dma_start(at_t[:], at_d[ds(u * F + kf * 128, 128), :])
                v_t = v_pool.tile([128, D], f32)
                nc.gpsimd.dma_start(v_t[:], v_d[ds(u * F + kf * 128, 128), :])
                nc.tensor.matmul(
                    psum[:], v_t[:], at_t[:], start=(kf == 0), stop=(kf == KF - 1)
                )
            o_t = o_pool.tile([D, S], f32)
            nc.any.tensor_copy(o_t[:], psum[:])
            nc.gpsimd.dma_start(o_d[ds(u * D, D), :], o_t[:])
    nc.compile()
    return nc


def _get_nc():
    if "nc" not in _cache:
        _cache["nc"] = _build_nc()
    return _cache["nc"]


def _topk_softmax(x, k):
    kth = np.partition(x, -k, axis=-1)[..., -k][..., None]
    keep = x >= kth
    e = np.exp(x - np.max(x, axis=-1, keepdims=True), dtype=np.float32) * keep
    return e / np.sum(e, axis=-1, keepdims=True, dtype=np.float32)


def _attn_mix(values, alpha, temp, gamma_hs, U, V, ln_w, ln_b):
    scale = np.float32(1.0 / np.sqrt(F))
    w = values.transpose(0, 2, 1, 3)  # [B,H,F,D]
    energy = np.mean(w * w, axis=-1, dtype=np.float32)  # [B,H,F]
    rms = np.maximum(
        np.sqrt(np.mean(energy, axis=-1, keepdims=True, dtype=np.float32)),
        np.float32(1e-6),
    )
    score = energy / rms
    gain = np.log1p(np.exp(temp, dtype=np.float32))[:, 0]  # softplus
    score = score * gain[None, :, None]
    mu = np.mean(score, axis=-1, keepdims=True, dtype=np.float32)
    var = np.mean((score - mu) ** 2, axis=-1, keepdims=True, dtype=np.float32)
    score = (score - mu) / np.sqrt(var + np.float32(LN_EPS)) * ln_w + ln_b
    bil = np.einsum("hsr,hrf->hsf", U, V).astype(np.float32)
    dl = score[:, :, None, :] + gamma_hs[None] + bil[None]  # [B,H,S,F]
    al = (alpha * scale)[None]  # [1,H,S,F]
    return _topk_softmax(dl, TOPK) + _topk_softmax(al, TOPK)


def kernel(**inputs):
    global last_result
    from concourse.bass_utils import run_bass_kernel_spmd

    values = np.ascontiguousarray(np.asarray(inputs["values"], dtype=np.float32))
    attn = _attn_mix(
        values,
        np.asarray(inputs["alpha"], np.float32),
        np.asarray(inputs["temp"], np.float32),
        np.asarray(inputs["gamma_hs"], np.float32),
        np.asarray(inputs["U"], np.float32),
        np.asarray(inputs["V"], np.float32),
        np.asarray(inputs["ln_w"], np.float32),
        np.asarray(inputs["ln_b"], np.float32),
    )  # [B,H,S,F]

    at_full = attn.transpose(0, 1, 3, 2)  # [B,H,F,S]
    vr = values.transpose(0, 2, 1, 3)  # [B,H,F,D]
    in_maps = []
    for i in range(NCORES):
        sl = slice(i * BPC, (i + 1) * BPC)
        in_maps.append(
            {
                "at": np.ascontiguousarray(at_full[sl]).reshape(BPC * H * F, S),
                "v": np.ascontiguousarray(vr[sl]).reshape(BPC * H * F, D),
            }
        )

    nc = _get_nc()
    import time as _time

    _t0 = _time.time()
    last_result = run_bass_kernel_spmd(nc, in_maps, core_ids=list(range(NCORES)))
    _cache["device_wall_s"] = _time.time() - _t0
    outs = []
    for i in range(NCORES):
        o = np.asarray(last_result.results[i]["o"]).reshape(BPC, H, D, S)
        outs.append(o.transpose(0, 3, 1, 2))  # [b,s,h,d]
    return np.ascontiguousarray(np.concatenate(outs, axis=0)).astype(np.float32)



# revision 12
# speedup vs baseline: 7.6968x; 7.6968x over previous
import sys

import numpy as np

sys.path.insert(0, "/opt/trn_rl_repo")

B, H, S, F, D = 16, 8, 512, 512, 64
RANK = 12
TOPK = 51  # int(0.1 * F)
LN_EPS = 1e-5
NCORES = 8
NEG = -1e30
KF = F // 128  # f chunks
KB = S // 128  # s blocks

_cache = {}
last_result = None  # test.py can inspect exec_time_ns / traces


def _build_nc():
    """SPMD program (identical on all 8 cores); core c handles head h=c.

    Device computes, per head: bilinear logits, data-path exact top-51
    (max8/match_replace), masked softmax, transposes and the
    attn @ values matmuls. The tiny fp32 score rows [B,F] and the
    batch-independent alpha softmax (precomputed fp32-exact, shipped f16
    pre-transposed) come from the host so top-k selection is bit-accurate.
    ~2.6MB in / ~1MB out per core crosses the axon tunnel.
    """
    from contextlib import ExitStack

    import concourse.mybir as mybir
    import concourse.tile as tile
    from concourse import bacc, masks
    from concourse.bass import ds

    nc = bacc.Bacc(
        "TRN2",
        target_bir_lowering=False,
        debug=False,
        num_devices=NCORES,
    )
    f32 = mybir.dt.float32
    f16 = mybir.dt.float16
    AF = mybir.ActivationFunctionType
    OP = mybir.AluOpType

    v_d = nc.dram_tensor("v", [B * F, D], f16, kind="ExternalInput").ap()
    at_d = nc.dram_tensor("at", [F, S], f16, kind="ExternalInput").ap()
    sc_d = nc.dram_tensor("sc", [B, F], f32, kind="ExternalInput").ap()
    ut_d = nc.dram_tensor("ut", [RANK, S], f32, kind="ExternalInput").ap()
    vv_d = nc.dram_tensor("vv", [RANK, F], f32, kind="ExternalInput").ap()
    gam_d = nc.dram_tensor("gam", [128, KB], f32, kind="ExternalInput").ap()
    o_d = nc.dram_tensor("o", [B * S, D], f16, kind="ExternalOutput").ap()

    with tile.TileContext(nc) as tc, ExitStack() as ctx:
        singles = ctx.enter_context(tc.tile_pool(name="si", bufs=1))
        small = ctx.enter_context(tc.tile_pool(name="sm", bufs=4))
        work = ctx.enter_context(tc.tile_pool(name="wk", bufs=8))
        et_pool = ctx.enter_context(tc.tile_pool(name="et", bufs=2))
        sc_pool = ctx.enter_context(tc.tile_pool(name="sc", bufs=2))
        o_pool = ctx.enter_context(tc.tile_pool(name="oo", bufs=4))
        em_pool = ctx.enter_context(tc.tile_pool(name="em", bufs=5))
        p_big = ctx.enter_context(tc.tile_pool(name="pb", bufs=1, space="PSUM"))
        p_tp = ctx.enter_context(tc.tile_pool(name="pt", bufs=4, space="PSUM"))
        p_mm = ctx.enter_context(tc.tile_pool(name="pm", bufs=2, space="PSUM"))

        ident = singles.tile([128, 128], f32)
        masks.make_identity(nc, ident[:])

        # ---- resident inputs ----
        v_sb = singles.tile([128, B, KF, D], f16)  # values, f on partitions
        for b in range(B):
            for kf in range(KF):
                nc.sync.dma_start(
                    v_sb[:, b, kf, :], v_d[ds(b * F + kf * 128, 128), :]
                )
        at16 = singles.tile([128, KF, S], f16)  # sm_alpha^T, f on partitions
        for kf in range(KF):
            nc.sync.dma_start(at16[:, kf, :], at_d[ds(kf * 128, 128), :])
        ut_sb = singles.tile([RANK, S], f32)
        nc.sync.dma_start(ut_sb[:], ut_d[:, :])
        vv_sb = singles.tile([RANK, F], f32)
        nc.sync.dma_start(vv_sb[:], vv_d[:, :])
        gam_sb = singles.tile([128, KB], f32)
        nc.sync.dma_start(gam_sb[:], gam_d[:, :])

        # ---- bilinear + gamma: bil[kb][p, f] (s on partitions) ----
        bil_sb = singles.tile([128, KB, F], f32)
        for kb in range(KB):
            bil_ps = p_big.tile([128, F], f32)
            nc.tensor.matmul(
                bil_ps[:], ut_sb[:, kb * 128 : (kb + 1) * 128], vv_sb[:]
            )
            nc.vector.tensor_scalar_add(
                bil_sb[:, kb, :], bil_ps[:], gam_sb[:, kb : kb + 1]
            )

        # ---- per-batch data path ----
        for b in range(B):
            sc_b = sc_pool.tile([128, F], f32)
            nc.sync.dma_start(sc_b[:], sc_d[b : b + 1, :].to_broadcast([128, F]))
            et16 = et_pool.tile([128, KF, S], f16)
            zd = small.tile([128, KB], f32)
            em_tiles = []
            for kb in range(KB):
                x = work.tile([128, F], f32)
                nc.gpsimd.tensor_add(x[:], bil_sb[:, kb, :], sc_b[:])
                # exact top-51: 7 rounds of max8 (+match_replace) on a copy
                wt = work.tile([128, F], f32)
                nc.scalar.copy(wt[:], x[:])
                m8 = work.tile([128, 7, 8], f32)
                for r in range(7):
                    nc.vector.max(m8[:, r, :], wt[:])
                    if r < 6:
                        nc.vector.match_replace(wt[:], m8[:, r, :], wt[:], NEG)
                nm = work.tile([128, 1], f32)
                nc.gpsimd.tensor_scalar_mul(nm[:], m8[:, 0, 0:1], -1.0)
                e = work.tile([128, F], f32)
                nc.scalar.activation(e[:], x[:], AF.Exp, bias=nm[:])
                # kth largest = rank 51 = m8[round 6][idx 2]
                e_m = em_pool.tile([128, F], f32)
                nc.vector.scalar_tensor_tensor(
                    e_m[:],
                    x[:],
                    m8[:, 6, 2:3],
                    e[:],
                    op0=OP.is_ge,
                    op1=OP.mult,
                    accum_out=zd[:, kb : kb + 1],
                )
                em_tiles.append(e_m)
            inv_zd = small.tile([128, KB], f32)
            nc.vector.reciprocal(inv_zd[:], zd[:])
            for kb in range(KB):
                e_m = em_tiles[kb]
                nc.gpsimd.tensor_scalar(
                    e_m[:], e_m[:], inv_zd[:, kb : kb + 1], None, op0=OP.mult
                )
                for kf in range(KF):
                    tp = p_tp.tile([128, 128], f32)
                    nc.tensor.transpose(
                        tp[:], e_m[:, kf * 128 : (kf + 1) * 128], ident[:]
                    )
                    nc.scalar.copy(
                        et16[:, kf, kb * 128 : (kb + 1) * 128], tp[:]
                    )
            for kb in range(KB):
                ps = p_mm.tile([128, D], f32)
                for kf in range(KF):
                    nc.tensor.matmul(
                        ps[:],
                        et16[:, kf, kb * 128 : (kb + 1) * 128],
                        v_sb[:, b, kf, :],
                        start=(kf == 0),
                        stop=False,
                    )
                for kf in range(KF):
                    nc.tensor.matmul(
                        ps[:],
                        at16[:, kf, kb * 128 : (kb + 1) * 128],
                        v_sb[:, b, kf, :],
                        start=False,
                        stop=(kf == KF - 1),
                    )
                o_sb = o_pool.tile([128, D], f16)
                nc.scalar.copy(o_sb[:], ps[:])
                nc.sync.dma_start(o_d[ds(b * S + kb * 128, 128), :], o_sb[:])
    nc.compile()
    return nc


def _get_nc():
    if "nc" not in _cache:
        _cache["nc"] = _build_nc()
    return _cache["nc"]


def make_in_maps(inputs):
    values = np.asarray(inputs["values"], dtype=np.float32)
    alpha = np.asarray(inputs["alpha"], np.float32)
    temp = np.asarray(inputs["temp"], np.float32)
    gamma_hs = np.asarray(inputs["gamma_hs"], np.float32)
    U = np.asarray(inputs["U"], np.float32)
    V = np.asarray(inputs["V"], np.float32)
    ln_w = np.asarray(inputs["ln_w"], np.float32)
    ln_b = np.asarray(inputs["ln_b"], np.float32)

    scale = np.float32(1.0 / np.sqrt(F))
    v16 = values.astype(np.float16)  # [B,F,H,D]

    # fp32-exact score rows (tiny): energy -> rms -> gain -> LN, all heads.
    w = values.transpose(0, 2, 1, 3)  # [B,H,F,D]
    energy = np.mean(w * w, axis=-1, dtype=np.float32)  # [B,H,F]
    rms = np.maximum(
        np.sqrt(np.mean(energy, axis=-1, keepdims=True, dtype=np.float32)),
        np.float32(1e-6),
    )
    gain = np.log1p(np.exp(temp, dtype=np.float32))[:, 0]  # softplus [H]
    score = energy / rms * gain[None, :, None]
    mu = np.mean(score, axis=-1, keepdims=True, dtype=np.float32)
    var = np.mean((score - mu) ** 2, axis=-1, keepdims=True, dtype=np.float32)
    score = (score - mu) / np.sqrt(var + np.float32(LN_EPS)) * ln_w + ln_b
    score = np.ascontiguousarray(score.transpose(1, 0, 2), dtype=np.float32)  # [H,B,F]

    # fp32-exact alpha top-k softmax (batch independent), shipped transposed f16.
    al = alpha * scale  # [H,S,F]
    kth = np.partition(al, F - TOPK, axis=-1)[..., F - TOPK][..., None]
    e = np.exp(al - np.max(al, axis=-1, keepdims=True), dtype=np.float32)
    e *= al >= kth
    sm_a = e / np.sum(e, axis=-1, keepdims=True, dtype=np.float32)
    smaT16 = np.ascontiguousarray(sm_a.transpose(0, 2, 1)).astype(np.float16)  # [H,F,S]

    in_maps = []
    for h in range(NCORES):
        in_maps.append(
            {
                "v": np.ascontiguousarray(v16[:, :, h, :]).reshape(B * F, D),
                "at": smaT16[h],
                "sc": score[h],
                "ut": np.ascontiguousarray(U[h].T),
                "vv": np.ascontiguousarray(V[h]),
                "gam": np.ascontiguousarray(gamma_hs[h, :, 0].reshape(KB, 128).T),
            }
        )
    return in_maps


def kernel(**inputs):
    global last_result
    from concourse.bass_utils import run_bass_kernel_spmd

    in_maps = make_in_maps(inputs)
    nc = _get_nc()
    import time as _time

    _t0 = _time.time()
    last_result = run_bass_kernel_spmd(nc, in_maps, core_ids=list(range(NCORES)))
    _cache["device_wall_s"] = _time.time() - _t0
    out = np.empty((B, S, H, D), dtype=np.float32)
    for h in range(NCORES):
        o = np.asarray(last_result.results[h]["o"]).reshape(B, S, D)
        out[:, :, h, :] = o.astype(np.float32)
    return out
